# revision 1
# baseline (speedup 1.0000x reference)
"""Causal self-attention Trainium2 kernel (8 NeuronCores, SPMD).

Sharding: 8 cores = 4 batches x 2 head-groups. Each core computes, for its
(batch b, head-group g): Q/K/V projections restricted to g's 8 heads
(column-parallel), causal attention for those heads, and the partial output
projection ctx_g @ Wo[g rows] (row-parallel). Host sums the two partials per
batch and adds the bias terms (bv @ Wo + bo).

All matmuls run in bf16 with fp32 PSUM accumulation. Attention uses the
transposed-scores orientation: scoresT[k, q] tiles are exp'd in place and fed
directly as the moving operand of the PV matmul (no PE transposes, all
matmuls N=512). Softmax skips the max subtraction (scores are ~N(0,1); exp
cannot overflow), folds the 1/sqrt(dh) scale into the exp activation, and
computes the normalizer as a partition-reduce on the otherwise-idle GpSimd
engine, broadcast back across partitions with a stride-0 DMA.
"""

import sys

sys.path.insert(0, "/opt/trn_rl_repo")

from contextlib import ExitStack

import numpy as np

import concourse.bass as bass
import concourse.tile as tile
from concourse import bass_isa, mybir
from concourse.bass_utils import run_bass_kernel_spmd

BF16 = mybir.dt.bfloat16
F32 = mybir.dt.float32
NP_BF16 = mybir.dt.np(BF16)

# Problem constants (hardcoded per contract).
B = 4          # batch
S = 2048       # sequence length
DM = 2048      # d_model
H = 16         # total heads
HD = 128       # head dim
G = 2          # head groups (tensor parallel degree)
NHL = H // G   # local heads per core
DHL = NHL * HD # local head dims
NCORES = 8
P = 128        # partitions
FD = 512       # matmul moving free dim (one PSUM bank of f32)
SCALE = 1.0 / float(np.sqrt(HD))
MASK_VAL = -1e30

# Opcodes whose walrus lowering handles multi-wait sync itself (or that we
# must not touch). Everything else gets its waits normalized to <= 1.
_WAIT_EXEMPT = {
    "NoOp",
    "EventSemaphore",
    "UnconditionalBranch",
    "RegisterMove",
    "ISA",
    "TileRelease",
}


def _fix_sync_waits(nc, max_waits=1):
    """Hoist extra sync-waits onto single-wait NoOps on the issuing engine.

    Several walrus instruction encodings (PSEUDO_DMA_DIRECT2D, S3_LW, CTRL_NO,
    ...) have a single sync-wait slot and fail codegen with "Too many sync
    wait commands" when Tile attaches more. A NoOp on the same engine
    immediately before the instruction performs the extra wait at the
    sequencer, which is semantically identical.
    """
    f = nc.m.functions[0]
    fixed = 0

    def walk(blocks):
        nonlocal fixed
        for b in blocks:
            il = b.instructions
            i = 0
            while i < len(il):
                inst = il[i]
                si = getattr(inst, "sync_info", None)
                ow = list(si.on_wait) if si is not None and si.on_wait else []
                if inst.opcode not in _WAIT_EXEMPT and len(ow) > max_waits:
                    keep = ow[len(ow) - max_waits :]
                    extra = ow[: len(ow) - max_waits]
                    for j, w in enumerate(extra):
                        nop = mybir.InstNoOp(
                            name=f"{inst.name}_waitfix{j}",
                            engine=inst.engine,
                            ins=[],
                            outs=[],
                            bass_nofuse=True,
                            sync_info=mybir.SyncInfo(on_wait=[w], on_update=[]),
                        )
                        il.insert(i, nop)
                        i += 1
                    inst.sync_info = mybir.SyncInfo(
                        on_wait=keep,
                        on_update=list(si.on_update) if si.on_update else [],
                    )
                    fixed += 1
                i += 1
            walk(getattr(b, "blocks", []) or [])

    walk(f.blocks)
    return fixed


def _bcast_ap(ap, nparts):
    """Partition-broadcast view of a single-partition AP."""
    return bass.AP(
        tensor=ap.tensor, offset=ap.offset, ap=[[0, nparts]] + list(ap.ap[1:])
    )


def build_nc(seq=S, dm=DM, nhl=NHL, fix_waits=True):
    """Build the single-core Bass program (same program for all 8 cores)."""
    dhl = nhl * P
    nkc = dm // P    # contraction chunks for projections
    nst = seq // P   # seq tiles
    nqb = seq // FD  # 512-wide q blocks

    nc = bass.Bass()
    # All inputs are pre-arranged on the host into SBUF-friendly layouts so
    # every DMA is contiguous per partition line.
    xT_d = nc.dram_tensor("xT", [P, nkc, seq], BF16, kind="ExternalInput")
    wq_d = nc.dram_tensor("wq", [nhl, P, nkc, P], BF16, kind="ExternalInput")
    wk_d = nc.dram_tensor("wk", [nhl, P, nkc, P], BF16, kind="ExternalInput")
    wv_d = nc.dram_tensor("wv", [P, nkc, dhl], BF16, kind="ExternalInput")
    wo_d = nc.dram_tensor("wo", [P, dhl // P, dm], BF16, kind="ExternalInput")
    bqk_d = nc.dram_tensor("bqk", [P, 2, nhl], F32, kind="ExternalInput")
    out_d = nc.dram_tensor("out", [seq, dm], F32, kind="ExternalOutput")

    with tile.TileContext(nc) as tc:
        es_qkv = ExitStack()
        qkv = es_qkv.enter_context(tc.tile_pool(name="qkv", bufs=1))
        QT = qkv.tile([P, nhl, seq], BF16)   # [hd-within-head, h, seq]
        KT = qkv.tile([P, nhl, seq], BF16)
        V = qkv.tile([P, nst, dhl], BF16)    # [seq-within-tile, st, dv]

        consts = es_qkv.enter_context(tc.tile_pool(name="consts", bufs=1))
        bqk_sb = consts.tile([P, 2, nhl], F32)
        ones_sb = consts.tile([P, 1], BF16)
        nc.vector.memset(ones_sb[:, :], 1.0)
        # Upper-keep mask for the transposed diagonal block:
        # umask[k, q] = 0 if q >= k else MASK_VAL.
        umask = consts.tile([P, P], F32)
        nc.gpsimd.memset(umask[:, :], 0.0)
        nc.gpsimd.affine_select(
            out=umask[:, :],
            in_=umask[:, :],
            compare_op=mybir.AluOpType.is_ge,
            fill=MASK_VAL,
            base=0,
            pattern=[[1, P]],
            channel_multiplier=-1,
        )
        nc.gpsimd.dma_start(out=bqk_sb[:, :, :], in_=bqk_d[:, :, :])

        # Strip pool opened early so h=0/1 strips can prefetch during P1.
        # Lives on the right-side stack, which is empty until phase 3.
        es_strip = ExitStack()
        spool = es_strip.enter_context(
            tc.tile_pool(name="spool", bufs=2, side="right")
        )
        strips = {}

        def load_strip(h):
            wqs = spool.tile([P, nkc, P], BF16, tag="wqs", name=f"wqs{h}")
            wks = spool.tile([P, nkc, P], BF16, tag="wks", name=f"wks{h}")
            nc.gpsimd.dma_start(out=wqs[:, :, :], in_=wq_d[h, :, :, :])
            nc.gpsimd.dma_start(out=wks[:, :, :], in_=wk_d[h, :, :, :])
            strips[h] = (wqs, wks)

        es_x = ExitStack()
        xpool = es_x.enter_context(tc.tile_pool(name="xpool", bufs=1))
        xT = xpool.tile([P, nkc, seq], BF16)
        for i in range(nkc):
            nc.gpsimd.dma_start(out=xT[:, i : i + 1, :], in_=xT_d[:, i : i + 1, :])


        load_strip(0)
        if nhl > 1:
            load_strip(1)

        es_ppsum = ExitStack()
        ppsum = es_ppsum.enter_context(
            tc.tile_pool(name="ppsum", bufs=8, space="PSUM")
        )
        # ---------------- Phase 2: QT = (x@Wq)^T, KT = (x@Wk)^T -------------
        nsc = seq // FD
        for h in range(nhl):
            wqs, wks = strips.pop(h)
            if h + 2 < nhl:
                load_strip(h + 2)
            for half in range(0, nsc, 2):
                nj = min(2, nsc - half)
                qps = [
                    ppsum.tile([P, FD], F32, tag="ppsum", bufs=8, name=f"qps{j}")
                    for j in range(nj)
                ]
                kps = [
                    ppsum.tile([P, FD], F32, tag="ppsum", bufs=8, name=f"kps{j}")
                    for j in range(nj)
                ]
                for c in range(nkc):
                    for j in range(nj):
                        sc = half + j
                        nc.tensor.matmul(
                            qps[j][:, :],
                            wqs[:, c, :],
                            xT[:, c, sc * FD : (sc + 1) * FD],
                            start=(c == 0),
                            stop=(c == nkc - 1),
                        )
                        nc.tensor.matmul(
                            kps[j][:, :],
                            wks[:, c, :],
                            xT[:, c, sc * FD : (sc + 1) * FD],
                            start=(c == 0),
                            stop=(c == nkc - 1),
                        )
                for j in range(nj):
                    sc = half + j
                    nc.scalar.activation(
                        QT[:, h, sc * FD : (sc + 1) * FD],
                        qps[j][:, :],
                        mybir.ActivationFunctionType.Identity,
                        bias=bqk_sb[:, 0, h : h + 1],
                    )
                    nc.scalar.activation(
                        KT[:, h, sc * FD : (sc + 1) * FD],
                        kps[j][:, :],
                        mybir.ActivationFunctionType.Identity,
                        bias=bqk_sb[:, 1, h : h + 1],
                    )
        es_strip.close()
        es_wv = ExitStack()
        wvpool = es_wv.enter_context(tc.tile_pool(name="wvpool", bufs=1))
        wv_sb = wvpool.tile([P, nkc, dhl], BF16)
        wstep = min(2, nkc)
        for i in range(0, nkc, wstep):
            nc.gpsimd.dma_start(
                out=wv_sb[:, i : i + wstep, :], in_=wv_d[:, i : i + wstep, :]
            )

        # ---------------- Phase 1b: V = x @ Wv  ([seq, dhl] layout) ---------
        fdv = min(FD, dhl)
        for st in range(nst):
            for dc in range(dhl // fdv):
                ps = ppsum.tile([P, fdv], F32, tag="ppsum", bufs=8)
                for c in range(nkc):
                    nc.tensor.matmul(
                        ps[:, :],
                        xT[:, c, st * P : (st + 1) * P],
                        wv_sb[:, c, dc * fdv : (dc + 1) * fdv],
                        start=(c == 0),
                        stop=(c == nkc - 1),
                    )
                nc.vector.tensor_copy(V[:, st, dc * fdv : (dc + 1) * fdv], ps[:, :])
        es_wv.close()
        es_ppsum.close()
        es_x.close()

        # ---------------- Phase 3: causal attention (transposed scores) ------
        # Per (h, qb): scoresT[k, qb*512:(qb+1)*512] per k-tile, exp in place,
        # PV consumes expT directly as the moving operand. Normalizer =
        # partition-reduce on GpSimd, reciprocal on DVE, stride-0 DMA
        # broadcast, multiply on evict.
        es_ctxT = ExitStack()
        ctxTpool = es_ctxT.enter_context(
            tc.tile_pool(name="ctxTpool", bufs=1, side="right")
        )
        ctxT = ctxTpool.tile([P, nhl, seq], BF16)
        # Prefetch wo during attention (right side, persists into P4).
        es_proj = ExitStack()
        wopool = es_proj.enter_context(
            tc.tile_pool(name="wopool", bufs=1, side="right")
        )
        wo_sb = wopool.tile([P, dhl // P, dm], BF16)
        ostep = min(2, dhl // P)
        for i in range(0, dhl // P, ostep):
            nc.gpsimd.dma_start(
                out=wo_sb[:, i : i + ostep, :], in_=wo_d[:, i : i + ostep, :]
            )

        es_attn = ExitStack()
        spsum = es_attn.enter_context(tc.tile_pool(name="spsum", bufs=4, space="PSUM"))
        cpsum = es_attn.enter_context(tc.tile_pool(name="cpsum", bufs=2, space="PSUM"))
        apool = es_attn.enter_context(tc.tile_pool(name="apool", bufs=2))
        npool = es_attn.enter_context(tc.tile_pool(name="npool", bufs=2))
        npsum = es_attn.enter_context(tc.tile_pool(name="npsum", bufs=2, space="PSUM"))
        dpool = es_attn.enter_context(tc.tile_pool(name="dpool", bufs=2, space="DRAM"))

        state = {}

        def stage_scores(h, qb):
            kmax = (qb + 1) * (FD // P)  # k-tiles for this q block
            exp_sb = apool.tile([P, nst, FD], BF16, tag="exp", name=f"exp{h}_{qb}")
            for kt in range(kmax):
                ps = spsum.tile([P, FD], F32, tag="spsum", bufs=4, name=f"sps{kt}")
                nc.tensor.matmul(
                    ps[:, :],
                    KT[:, h, kt * P : (kt + 1) * P],
                    QT[:, h, qb * FD : (qb + 1) * FD],
                    start=True,
                    stop=True,
                )
                j = kt - 4 * qb
                if j >= 0:
                    # diagonal block: keep q >= k within the block
                    nc.vector.tensor_add(
                        ps[:, j * P : (j + 1) * P],
                        ps[:, j * P : (j + 1) * P],
                        umask[:, :],
                    )
                nc.scalar.activation(
                    exp_sb[:, kt, :],
                    ps[:, :],
                    mybir.ActivationFunctionType.Exp,
                    scale=SCALE,
                )
                if j > 0:
                    # q < k region of partial diagonal tiles: attn weight 0
                    nc.vector.memset(exp_sb[:, kt, : j * P], 0.0)
            state[(h, qb)] = (exp_sb, kmax)

        def stage_pv(h, qb):
            exp_sb, kmax = state.pop((h, qb))
            pv = cpsum.tile([P, FD], F32, tag="pv", bufs=2, name=f"pv{h}_{qb}")
            csum = npsum.tile([1, FD], F32, tag="csum", bufs=2, name=f"cs{h}_{qb}")
            for kt in range(kmax):
                nc.tensor.matmul(
                    pv[:, :],
                    V[:, kt, h * P : (h + 1) * P],
                    exp_sb[:, kt, :],
                    start=(kt == 0),
                    stop=(kt == kmax - 1),
                )
                nc.tensor.matmul(
                    csum[:, :],
                    ones_sb[:, :],
                    exp_sb[:, kt, :],
                    start=(kt == 0),
                    stop=(kt == kmax - 1),
                )
            recip = npool.tile([1, FD], F32, tag="recip", name=f"recip{h}_{qb}")
            bc = npool.tile([P, FD], F32, tag="bc", name=f"bc{h}_{qb}")
            nc.vector.reciprocal(recip[:, :], csum[0:1, :])
            # Partition-broadcast via a DRAM bounce (zero partition step is
            # only legal on DRAM APs).
            rd = dpool.tile([1, FD], F32, tag="rd", name=f"rd{h}_{qb}")
            nc.sync.dma_start(out=rd[:, :], in_=recip[:, :])
            nc.sync.dma_start(out=bc[:, :], in_=_bcast_ap(rd[:, :], P))
            nc.vector.tensor_mul(
                ctxT[:, h, qb * FD : (qb + 1) * FD], pv[:, :], bc[:, :]
            )

        prev = None
        for h in range(nhl):
            for qb in range(nqb):
                stage_scores(h, qb)
                if prev is not None:
                    stage_pv(*prev)
                prev = (h, qb)
        stage_pv(*prev)
        es_attn.close()
        es_qkv.close()

        # ---------------- Phase 4: out = ctx @ Wo ---------------------------
        opsum = es_proj.enter_context(tc.tile_pool(name="opsum", bufs=4, space="PSUM"))
        opool = es_proj.enter_context(
            tc.tile_pool(name="opool", bufs=4, side="right")
        )
        for st in range(nst):
            for mc in range(dm // FD):
                ps = opsum.tile([P, FD], F32, tag="ops", bufs=4)
                for dc in range(dhl // P):
                    nc.tensor.matmul(
                        ps[:, :],
                        ctxT[:, dc, st * P : (st + 1) * P],
                        wo_sb[:, dc, mc * FD : (mc + 1) * FD],
                        start=(dc == 0),
                        stop=(dc == dhl // P - 1),
                    )
                ot = opool.tile([P, FD], F32, tag="ot")
                nc.scalar.copy(ot[:, :], ps[:, :])
                nc.sync.dma_start(
                    out=out_d[st * P : (st + 1) * P, mc * FD : (mc + 1) * FD],
                    in_=ot[:, :],
                )
        es_proj.close()
        es_ctxT.close()

    if fix_waits:
        _fix_sync_waits(nc)
    return nc


def shard_inputs(x, Wq, bq, Wk, bk, Wv, bv, Wo, bo, seq=S, dm=DM, nhl=NHL, nb=B, g_=G):
    """Host-side sharding: returns per-core input maps (bf16 pre-arranged)."""
    dhl = nhl * P
    nkc = dm // P
    xTs = []
    for b in range(nb):
        xt = np.ascontiguousarray(x[b].T).astype(NP_BF16)  # [dm, seq]
        xTs.append(np.ascontiguousarray(xt.reshape(nkc, P, seq).transpose(1, 0, 2)))
    wqs, wks, wvs, wos, bqks = [], [], [], [], []
    for g in range(g_):
        sl = slice(g * dhl, (g + 1) * dhl)
        wq_s = Wq[:, sl].astype(NP_BF16)
        wk_s = Wk[:, sl].astype(NP_BF16)
        wv_s = Wv[:, sl].astype(NP_BF16)
        wo_s = Wo[sl, :].astype(NP_BF16)
        # wq/wk: [nhl, P, nkc, P] strip-major
        wqs.append(
            np.ascontiguousarray(wq_s.reshape(nkc, P, nhl, P).transpose(2, 1, 0, 3))
        )
        wks.append(
            np.ascontiguousarray(wk_s.reshape(nkc, P, nhl, P).transpose(2, 1, 0, 3))
        )
        wvs.append(np.ascontiguousarray(wv_s.reshape(nkc, P, dhl).transpose(1, 0, 2)))
        wos.append(
            np.ascontiguousarray(wo_s.reshape(dhl // P, P, dm).transpose(1, 0, 2))
        )
        bqk = np.stack(
            [
                np.asarray(bq[sl], np.float32).reshape(nhl, P),
                np.asarray(bk[sl], np.float32).reshape(nhl, P),
            ]
        )  # [2, nhl, P]
        bqks.append(np.ascontiguousarray(bqk.transpose(2, 0, 1)))  # [P, 2, nhl]
    in_maps = []
    for c in range(nb * g_):
        b, g = divmod(c, g_)
        in_maps.append(
            {
                "xT": xTs[b],
                "wq": wqs[g],
                "wk": wks[g],
                "wv": wvs[g],
                "wo": wos[g],
                "bqk": bqks[g],
            }
        )
    return in_maps


_CACHE = {}


def _get_nc():
    if "nc" not in _CACHE:
        _CACHE["nc"] = build_nc()
    return _CACHE["nc"]


def run(inputs, trace=False):
    """Run the SPMD kernel; returns (full_output, BassKernelResults)."""
    inputs = {k: np.asarray(v) for k, v in inputs.items()}
    nc = _get_nc()
    in_maps = shard_inputs(**inputs)
    res = run_bass_kernel_spmd(
        nc, in_maps, core_ids=list(range(NCORES)), trace=trace
    )
    Wo = np.asarray(inputs["Wo"], np.float32)
    const_row = (
        np.asarray(inputs["bv"], np.float32) @ Wo + np.asarray(inputs["bo"], np.float32)
    )
    out = np.empty((B, S, DM), np.float32)
    for b in range(B):
        out[b] = res.results[G * b]["out"] + res.results[G * b + 1]["out"] + const_row
    return out, res


def kernel(**inputs):
    out, _ = run(inputs, trace=False)
    return out

